# revision 1
# baseline (speedup 1.0000x reference)
"""Trainium2 Bass kernel for nn_Decoder: batched greedy autoregressive GRU decode.

Strategy (8 NeuronCores):
  - Vocab-shard the [V,H] classifier: core c holds Wc rows [c*6283, c*6283+w_c)
    (w_c = 6283 for c<7, 6276 for c=7), padded to 6656 cols, laid out as two
    halves on the 128 SBUF partitions: rows 0:64 = batch x vocab[0:3584),
    rows 64:128 = batch x vocab[3584:6656).
  - Replicate the small GRU; every core computes the identical hidden state.
  - Per step: classifier matmuls (col-packed via tile_position), bias-add,
    vector.max + max_index for per-core argmax candidates, tiny AllGather of
    (val, idx) pairs, lane-aligned global combine, indirect-DMA embedding
    gather, GRU step, PE transposes for h^T.
  - Output: each core writes its [L, B, 6656] logits shard; host concatenates.
"""

import sys

sys.path.insert(0, "/opt/trn_rl_repo")

import numpy as np

import concourse.bacc as bacc
import concourse.bass as bass
import concourse.mybir as mybir
import concourse.tile as tile
from concourse.bass_utils import run_bass_kernel_spmd
from concourse.masks import make_identity

F32 = mybir.dt.float32
I32 = mybir.dt.int32
U32 = mybir.dt.uint32


def _install_neff_cache():
    """Content-hash cache for compiled NEFFs (compile is ~minutes)."""
    try:
        import hashlib
        import os
        import shutil

        import concourse.bass2jax as b2j

        if getattr(b2j, "_neff_cache_installed", False):
            return
        orig = b2j.compile_bir_kernel

        def cached(bir_json, tmpdir, neff_name="file.neff"):
            try:
                h = hashlib.sha256(bir_json).hexdigest()
                cdir = "/tmp/neff_cache"
                os.makedirs(cdir, exist_ok=True)
                cpath = f"{cdir}/{h}.neff"
                if os.path.exists(cpath):
                    dst = os.path.join(tmpdir, neff_name)
                    shutil.copy(cpath, dst)
                    return dst
                out = orig(bir_json, tmpdir, neff_name=neff_name)
                shutil.copy(out, cpath)
                return out
            except Exception:
                return orig(bir_json, tmpdir, neff_name=neff_name)

        b2j.compile_bir_kernel = cached
        b2j._neff_cache_installed = True
    except Exception:
        pass


_install_neff_cache()

V = 50257
E = 256
H = 512
B = 64
L = 64
SOS = 2
N_CORES = 8

VS = 6656           # padded vocab shard width (13 chunks of 512)
HALF0 = 3584        # 7 chunks
HALF1 = 3072        # 6 chunks
W_BASE = 6283       # real shard width for cores 0..6 (core 7: 50257-7*6283=6276)
BIG = float(2 ** 16)   # > max idx (50636); keeps idx-BIG exact in f32
NEG = -1.0e30


def shard_width(c):
    return W_BASE if c < N_CORES - 1 else V - W_BASE * (N_CORES - 1)


def build_kernel(n_steps=L, sim_local=False):
    nc = bacc.Bacc("TRN2", target_bir_lowering=False, debug=False,
                   num_devices=1 if sim_local else N_CORES)

    # ---- DRAM I/O (identical program on all cores; per-core DATA differs) ----
    emb_d = nc.dram_tensor("emb", [V, E], F32, kind="ExternalInput")
    wihT_d = nc.dram_tensor("wihT", [E, 3 * H], F32, kind="ExternalInput")
    whhT_d = nc.dram_tensor("whhT", [H, 3 * H], F32, kind="ExternalInput")
    wcT_d = nc.dram_tensor("wcT", [H, VS], F32, kind="ExternalInput")
    bc2_d = nc.dram_tensor("bc2", [128, HALF0], F32, kind="ExternalInput")
    brz_d = nc.dram_tensor("brz", [B, 2 * H], F32, kind="ExternalInput")
    bihn_d = nc.dram_tensor("bihn", [B, H], F32, kind="ExternalInput")
    bhhn_d = nc.dram_tensor("bhhn", [B, H], F32, kind="ExternalInput")
    off_d = nc.dram_tensor("off128", [128, 1], F32, kind="ExternalInput")
    h0_d = nc.dram_tensor("h0s", [B, H], F32, kind="ExternalInput")
    h0T_d = nc.dram_tensor("h0T", [H, B], F32, kind="ExternalInput")

    out_d = nc.dram_tensor("out", [n_steps, B, VS], F32, kind="ExternalOutput")
    toks_d = nc.dram_tensor("toks", [n_steps, B, 1], I32, kind="ExternalOutput")

    with tile.TileContext(nc) as tc:
        with (
            tc.tile_pool(name="wpool", bufs=1) as wpool,
            tc.tile_pool(name="state", bufs=1) as state,
            tc.tile_pool(name="work", bufs=1) as work,
            tc.tile_pool(name="lgt", bufs=1) as lgt,
            tc.tile_pool(name="ps_cls", bufs=2, space="PSUM") as ps_cls,
            tc.tile_pool(name="ps_g", bufs=1, space="PSUM") as ps_g,
            tc.tile_pool(name="ps_t", bufs=2, space="PSUM") as ps_t,
            tc.tile_pool(name="dram", bufs=1, space="DRAM") as dram,
        ):
            # ---- resident weights ----
            wihT = wpool.tile([128, 2 * 3 * H], F32)          # 2 chunks of [128,1536]
            for k in range(2):
                nc.sync.dma_start(wihT[:, k * 1536:(k + 1) * 1536],
                                  wihT_d[128 * k:128 * (k + 1), :])
            whhT = wpool.tile([128, 4 * 3 * H], F32)          # 4 chunks
            for k in range(4):
                nc.sync.dma_start(whhT[:, k * 1536:(k + 1) * 1536],
                                  whhT_d[128 * k:128 * (k + 1), :])
            wcT = wpool.tile([128, 4 * VS], F32)              # 4 chunks of [128,6656]
            for k in range(4):
                nc.sync.dma_start(wcT[:, k * VS:(k + 1) * VS],
                                  wcT_d[128 * k:128 * (k + 1), :])
            bc2 = wpool.tile([128, HALF0], F32)
            nc.sync.dma_start(bc2[:], bc2_d[:])
            brz = wpool.tile([B, 2 * H], F32)
            nc.sync.dma_start(brz[:], brz_d[:])
            bihn = wpool.tile([B, H], F32)
            nc.sync.dma_start(bihn[:], bihn_d[:])
            bhhn = wpool.tile([B, H], F32)
            nc.sync.dma_start(bhhn[:], bhhn_d[:])
            off128 = wpool.tile([128, 1], F32)
            nc.sync.dma_start(off128[:], off_d[:])
            ident = wpool.tile([128, 128], F32)
            make_identity(nc, ident[:])

            # ---- persistent state ----
            h = state.tile([B, H], F32)
            nc.sync.dma_start(h[:], h0_d[:])
            hT = state.tile([128, 4 * B], F32)                # 4 chunks of [128,64]
            for k in range(4):
                nc.sync.dma_start(hT[:, B * k:B * (k + 1)],
                                  h0T_d[128 * k:128 * (k + 1), :])
            tok = state.tile([B, 1], I32)
            nc.vector.memset(tok[:], SOS)

            sb_logits = lgt.tile([128, HALF0], F32)
            # rows 64:128, cols HALF1:HALF0 are never written per-step; make
            # them a -inf pad so the max scan can cover the full [128, 3584].
            nc.vector.memset(sb_logits[64:128, HALF1:HALF0], NEG)

            agr = state.tile([N_CORES, 256], F32)
            if sim_local:
                nc.vector.memset(agr[1:N_CORES, :], NEG)

            for t in range(n_steps):
                if not sim_local:
                    # AG bounce buffers (DRAM) -- shared output needs one
                    # writer, so allocate per step
                    ag_in = dram.tile([1, 256], F32, tag=f"agi{t}")
                    ag_out_sh = dram.tile([N_CORES, 256], F32,
                                          addr_space="Shared", tag=f"ago{t}")

                # ---- 1. embedding gather: x[b] = emb[tok[b]] ----
                x = work.tile([B, E], F32, tag="x")
                nc.gpsimd.indirect_dma_start(
                    out=x[:], out_offset=None, in_=emb_d[:],
                    in_offset=bass.IndirectOffsetOnAxis(ap=tok[:, :1], axis=0),
                )
                # xT chunks [128, 64] x2 via PE transpose
                xT = work.tile([128, 2 * B], F32, tag="xT")
                for j in range(2):
                    pt = ps_t.tile([128, B], F32, tag="pst")
                    nc.tensor.transpose(out=pt[:], in_=x[:, 128 * j:128 * (j + 1)],
                                        identity=ident[0:B, 0:B])
                    nc.vector.tensor_copy(xT[:, B * j:B * (j + 1)], pt[:])

                # ---- 2. GRU matmuls ----
                # psum_g[:, 0:1024]  = gi_rz + gh_rz ; psum_g[:, 1024:1536] = gi_n
                # psum_hn            = gh_n
                pg = ps_g.tile([B, 3 * H], F32, tag="pg")
                phn = ps_g.tile([B, H], F32, tag="phn")
                for seg in range(3):
                    sl = slice(512 * seg, 512 * (seg + 1))
                    for k in range(2):
                        nc.tensor.matmul(
                            pg[:, sl], lhsT=xT[:, B * k:B * (k + 1)],
                            rhs=wihT[:, 1536 * k + 512 * seg:1536 * k + 512 * (seg + 1)],
                            start=(k == 0), stop=(seg == 2 and k == 1),
                        )
                    if seg < 2:
                        for k in range(4):
                            nc.tensor.matmul(
                                pg[:, sl], lhsT=hT[:, B * k:B * (k + 1)],
                                rhs=whhT[:, 1536 * k + 512 * seg:1536 * k + 512 * (seg + 1)],
                                start=False, stop=(k == 3),
                            )
                for k in range(4):
                    nc.tensor.matmul(
                        phn[:], lhsT=hT[:, B * k:B * (k + 1)],
                        rhs=whhT[:, 1536 * k + 1024:1536 * k + 1536],
                        start=(k == 0), stop=(k == 3),
                    )

                # ---- 3. GRU elementwise ----
                rz_pre = work.tile([B, 2 * H], F32, tag="rzp")
                nc.vector.tensor_add(rz_pre[:], pg[:, 0:1024], brz[:])
                rz = work.tile([B, 2 * H], F32, tag="rz")
                nc.scalar.activation(rz[:], rz_pre[:],
                                     mybir.ActivationFunctionType.Sigmoid)
                hn_b = work.tile([B, H], F32, tag="hnb")
                nc.vector.tensor_add(hn_b[:], phn[:], bhhn[:])
                tmp = work.tile([B, H], F32, tag="tmp")
                nc.vector.tensor_mul(tmp[:], rz[:, 0:512], hn_b[:])
                in_b = work.tile([B, H], F32, tag="inb")
                nc.vector.tensor_add(in_b[:], pg[:, 1024:1536], bihn[:])
                tmp2 = work.tile([B, H], F32, tag="tmp2")
                nc.vector.tensor_add(tmp2[:], tmp[:], in_b[:])
                ngate = work.tile([B, H], F32, tag="ngate")
                nc.scalar.activation(ngate[:], tmp2[:],
                                     mybir.ActivationFunctionType.Tanh)
                d = work.tile([B, H], F32, tag="d")
                nc.vector.tensor_sub(d[:], h[:], ngate[:])
                zd = work.tile([B, H], F32, tag="zd")
                nc.vector.tensor_mul(zd[:], rz[:, 512:1024], d[:])
                nc.vector.tensor_add(h[:], ngate[:], zd[:])

                # hT update: 4 PE transposes of h chunks
                for k in range(4):
                    pt = ps_t.tile([128, B], F32, tag="pst")
                    nc.tensor.transpose(out=pt[:], in_=h[:, 128 * k:128 * (k + 1)],
                                        identity=ident[0:B, 0:B])
                    nc.vector.tensor_copy(hT[:, B * k:B * (k + 1)], pt[:])

                # ---- 4. classifier: 7 chunk pairs (p=6: half0 solo) ----
                for p in range(7):
                    ps = ps_cls.tile([128, 512], F32, tag="cls")
                    for k in range(4):
                        nc.tensor.matmul(
                            ps[0:64, :], lhsT=hT[:, B * k:B * (k + 1)],
                            rhs=wcT[:, VS * k + 512 * p:VS * k + 512 * (p + 1)],
                            start=(k == 0), stop=(k == 3), tile_position=(0, 0),
                        )
                    if p < 6:
                        for k in range(4):
                            nc.tensor.matmul(
                                ps[64:128, :], lhsT=hT[:, B * k:B * (k + 1)],
                                rhs=wcT[:, VS * k + HALF0 + 512 * p:
                                        VS * k + HALF0 + 512 * (p + 1)],
                                start=(k == 0), stop=(k == 3), tile_position=(0, 64),
                            )
                        nc.vector.tensor_add(
                            sb_logits[:, 512 * p:512 * (p + 1)], ps[:],
                            bc2[:, 512 * p:512 * (p + 1)])
                    else:
                        nc.vector.tensor_add(
                            sb_logits[0:64, 512 * p:512 * (p + 1)], ps[0:64, :],
                            bc2[0:64, 512 * p:512 * (p + 1)])

                # ---- 5. DMA logits out ----
                nc.sync.dma_start(out_d[t, :, 0:HALF0], sb_logits[0:64, :])
                nc.sync.dma_start(out_d[t, :, HALF0:VS], sb_logits[64:128, 0:HALF1])
                nc.sync.dma_start(toks_d[t], tok[:])

                # ---- 6. local argmax candidates ----
                m8 = work.tile([128, 8], F32, tag="m8")
                mi8 = work.tile([128, 8], U32, tag="mi8")
                nc.vector.max(out=m8[:], in_=sb_logits[:])
                nc.vector.max_index(out=mi8[:], in_max=m8[:], in_values=sb_logits[:])
                vi = work.tile([128, 2], F32, tag="vi")
                nc.vector.tensor_copy(vi[:, 0:1], m8[:, 0:1])
                idxf = work.tile([128, 1], F32, tag="idxf")
                nc.vector.tensor_copy(idxf[:], mi8[:, 0:1])      # u32 -> f32
                nc.vector.tensor_add(vi[:, 1:2], idxf[:], off128[:])

                # ---- 7. AllGather of (val, idx) ----
                pvt = ps_t.tile([2, 128], F32, tag="pst")
                nc.tensor.transpose(out=pvt[:], in_=vi[:], identity=ident[:])
                agi = work.tile([2, 128], F32, tag="agi")
                nc.vector.tensor_copy(agi[:], pvt[:])
                if sim_local:
                    # timing-only stand-in for the collective round trip
                    nc.sync.dma_start(agr[0:1, :], agi[:])
                else:
                    nc.sync.dma_start(ag_in[:], agi[:])
                    nc.gpsimd.collective_compute(
                        "AllGather", mybir.AluOpType.bypass,
                        replica_groups=[list(range(N_CORES))],
                        ins=[ag_in.opt()], outs=[ag_out_sh.opt()],
                    )
                    nc.sync.dma_start(agr[:], ag_out_sh[:])

                # ---- 8. lane-aligned global combine ----
                # agr row c = [vA 64 | vB 64 | iA 64 | iB 64] from core c.
                # 4 transposes -> cand [64, 32] = [vA 8 | vB 8 | iA 8 | iB 8]
                pcand = ps_t.tile([B, 32], F32, tag="pst")
                for q in range(4):
                    nc.tensor.transpose(out=pcand[:, 8 * q:8 * (q + 1)],
                                        in_=agr[:, 64 * q:64 * (q + 1)],
                                        identity=ident[0:N_CORES, 0:N_CORES])
                cand = work.tile([B, 32], F32, tag="cand")
                nc.vector.tensor_copy(cand[:], pcand[:])
                gmax = work.tile([B, 1], F32, tag="gmax")
                nc.vector.tensor_reduce(gmax[:], cand[:, 0:16],
                                        axis=mybir.AxisListType.X,
                                        op=mybir.AluOpType.max)
                mask = work.tile([B, 16], F32, tag="mask")
                nc.vector.tensor_scalar(out=mask[:], in0=cand[:, 0:16],
                                        scalar1=gmax[:, 0:1], scalar2=None,
                                        op0=mybir.AluOpType.is_ge)
                idxmb = work.tile([B, 16], F32, tag="idxmb")
                nc.vector.tensor_scalar(out=idxmb[:], in0=cand[:, 16:32],
                                        scalar1=BIG, scalar2=None,
                                        op0=mybir.AluOpType.subtract)
                sel = work.tile([B, 16], F32, tag="sel")
                nc.vector.tensor_mul(sel[:], mask[:], idxmb[:])
                mn = work.tile([B, 1], F32, tag="mn")
                nc.vector.tensor_reduce(mn[:], sel[:],
                                        axis=mybir.AxisListType.X,
                                        op=mybir.AluOpType.min)
                tokf = work.tile([B, 1], F32, tag="tokf")
                # token = min(mn + BIG, V-1): clamp guards the emb gather
                nc.vector.tensor_scalar(out=tokf[:], in0=mn[:], scalar1=BIG,
                                        scalar2=float(V - 1),
                                        op0=mybir.AluOpType.add,
                                        op1=mybir.AluOpType.min)
                nc.vector.tensor_copy(tok[:], tokf[:])           # f32 -> i32

    nc.compile()
    return nc


def prep_inputs(h0, emb, W_ih, W_hh, b_ih, b_hh, Wc, bc):
    """Host-side numpy prep. Returns list of per-core input dicts."""
    h0 = np.asarray(h0, np.float32)
    emb = np.ascontiguousarray(np.asarray(emb, np.float32))
    W_ih = np.asarray(W_ih, np.float32)
    W_hh = np.asarray(W_hh, np.float32)
    b_ih = np.asarray(b_ih, np.float32)
    b_hh = np.asarray(b_hh, np.float32)
    Wc = np.asarray(Wc, np.float32)
    bc = np.asarray(bc, np.float32)

    wihT = np.ascontiguousarray(W_ih.T)                  # [256, 1536]
    whhT = np.ascontiguousarray(W_hh.T)                  # [512, 1536]
    brz = np.broadcast_to((b_ih + b_hh)[None, 0:1024], (B, 1024)).copy()
    bihn = np.broadcast_to(b_ih[None, 1024:1536], (B, 512)).copy()
    bhhn = np.broadcast_to(b_hh[None, 1024:1536], (B, 512)).copy()
    h0s = np.ascontiguousarray(h0[0])                    # [64, 512]
    h0T = np.ascontiguousarray(h0s.T)                    # [512, 64]

    in_maps = []
    for c in range(N_CORES):
        base = c * W_BASE
        w_c = shard_width(c)
        wcT = np.zeros((H, VS), np.float32)
        wcT[:, :w_c] = Wc[base:base + w_c].T
        bcp = np.full(VS, NEG, np.float32)
        bcp[:w_c] = bc[base:base + w_c]
        bc2 = np.empty((128, HALF0), np.float32)
        bc2[0:64, :] = bcp[0:HALF0][None, :]
        bc2[64:128, 0:HALF1] = bcp[HALF0:VS][None, :]
        bc2[64:128, HALF1:HALF0] = NEG
        off = np.empty((128, 1), np.float32)
        off[0:64] = base
        off[64:128] = base + HALF0
        in_maps.append(dict(
            emb=emb, wihT=wihT, whhT=whhT, wcT=np.ascontiguousarray(wcT),
            bc2=bc2, brz=brz, bihn=bihn, bhhn=bhhn, off128=off,
            h0s=h0s, h0T=h0T,
        ))
    return in_maps


_NC_CACHE = {}


def _get_nc(n_steps):
    if n_steps not in _NC_CACHE:
        _NC_CACHE[n_steps] = build_kernel(n_steps)
    return _NC_CACHE[n_steps]


def run(inputs, n_steps=L, trace=False):
    in_maps = prep_inputs(
        inputs["h0"], inputs["emb"], inputs["W_ih"], inputs["W_hh"],
        inputs["b_ih"], inputs["b_hh"], inputs["Wc"], inputs["bc"])
    nc = _get_nc(n_steps)
    res = run_bass_kernel_spmd(nc, in_maps, core_ids=list(range(N_CORES)),
                               trace=trace)
    parts = []
    for c in range(N_CORES):
        parts.append(res.results[c]["out"][:, :, :shard_width(c)])
    full = np.concatenate(parts, axis=2)
    toks = res.results[0]["toks"][:, :, 0]
    return full, toks, res


def kernel(**inputs) -> np.ndarray:
    n_steps = int(inputs.get("len_seq", L))
    full, _toks, _res = run(inputs, n_steps=n_steps)
    return full


if __name__ == "__main__":
    pass



# revision 6
# speedup vs baseline: 1.0026x; 1.0026x over previous
"""Trainium2 Bass kernel for nn_Decoder: batched greedy autoregressive GRU decode.

Strategy (8 NeuronCores):
  - Vocab-shard the [V,H] classifier: core c holds Wc rows [c*6283, c*6283+w_c)
    (w_c = 6283 for c<7, 6276 for c=7), padded to 6656 cols, laid out as two
    halves on the 128 SBUF partitions: rows 0:64 = batch x vocab[0:3584),
    rows 64:128 = batch x vocab[3584:6656).
  - Replicate the small GRU; every core computes the identical hidden state.
  - Per step: classifier matmuls (col-packed via tile_position), bias-add,
    vector.max + max_index for per-core argmax candidates, tiny AllGather of
    (val, idx) pairs, lane-aligned global combine, indirect-DMA embedding
    gather, GRU step, PE transposes for h^T.
  - Output: each core writes its [L, B, 6656] logits shard; host concatenates.
"""

import sys

sys.path.insert(0, "/opt/trn_rl_repo")

import numpy as np

import concourse.bacc as bacc
import concourse.bass as bass
import concourse.mybir as mybir
import concourse.tile as tile
from concourse.bass_utils import run_bass_kernel_spmd
from concourse.masks import make_identity

F32 = mybir.dt.float32
I32 = mybir.dt.int32
U32 = mybir.dt.uint32


def _install_neff_cache():
    """Content-hash cache for compiled NEFFs (compile is ~minutes)."""
    try:
        import hashlib
        import os
        import shutil

        import concourse.bass2jax as b2j

        if getattr(b2j, "_neff_cache_installed", False):
            return
        orig = b2j.compile_bir_kernel

        def cached(bir_json, tmpdir, neff_name="file.neff"):
            try:
                h = hashlib.sha256(bir_json).hexdigest()
                cdir = "/tmp/neff_cache"
                os.makedirs(cdir, exist_ok=True)
                cpath = f"{cdir}/{h}.neff"
                if os.path.exists(cpath):
                    dst = os.path.join(tmpdir, neff_name)
                    shutil.copy(cpath, dst)
                    return dst
                out = orig(bir_json, tmpdir, neff_name=neff_name)
                shutil.copy(out, cpath)
                return out
            except Exception:
                return orig(bir_json, tmpdir, neff_name=neff_name)

        b2j.compile_bir_kernel = cached
        b2j._neff_cache_installed = True
    except Exception:
        pass


_install_neff_cache()

V = 50257
E = 256
H = 512
B = 64
L = 64
SOS = 2
N_CORES = 8

VS = 6656           # padded vocab shard width (13 chunks of 512)
HALF0 = 3584        # 7 chunks
HALF1 = 3072        # 6 chunks
W_BASE = 6283       # real shard width for cores 0..6 (core 7: 50257-7*6283=6276)
BIG = float(2 ** 16)   # > max idx (50636); keeps idx-BIG exact in f32
NEG = -1.0e30


def shard_width(c):
    return W_BASE if c < N_CORES - 1 else V - W_BASE * (N_CORES - 1)


def build_kernel(n_steps=L, sim_local=False):
    nc = bacc.Bacc("TRN2", target_bir_lowering=False, debug=False,
                   num_devices=1 if sim_local else N_CORES)

    # ---- DRAM I/O (identical program on all cores; per-core DATA differs) ----
    emb_d = nc.dram_tensor("emb", [V, E], F32, kind="ExternalInput")
    wihT_d = nc.dram_tensor("wihT", [E, 3 * H], F32, kind="ExternalInput")
    whhT_d = nc.dram_tensor("whhT", [H, 3 * H], F32, kind="ExternalInput")
    wcT_d = nc.dram_tensor("wcT", [H, VS], F32, kind="ExternalInput")
    bc2_d = nc.dram_tensor("bc2", [128, HALF0], F32, kind="ExternalInput")
    brz_d = nc.dram_tensor("brz", [B, 2 * H], F32, kind="ExternalInput")
    bihn_d = nc.dram_tensor("bihn", [B, H], F32, kind="ExternalInput")
    bhhn_d = nc.dram_tensor("bhhn", [B, H], F32, kind="ExternalInput")
    off_d = nc.dram_tensor("off128", [128, 1], F32, kind="ExternalInput")
    h0_d = nc.dram_tensor("h0s", [B, H], F32, kind="ExternalInput")
    h0T_d = nc.dram_tensor("h0T", [H, B], F32, kind="ExternalInput")

    out_d = nc.dram_tensor("out", [n_steps, B, VS], F32, kind="ExternalOutput")
    toks_d = nc.dram_tensor("toks", [n_steps, B, 1], I32, kind="ExternalOutput")

    with tile.TileContext(nc) as tc:
        with (
            tc.tile_pool(name="wpool", bufs=1) as wpool,
            tc.tile_pool(name="state", bufs=1) as state,
            tc.tile_pool(name="work", bufs=1) as work,
            tc.tile_pool(name="lgt", bufs=1) as lgt,
            tc.tile_pool(name="ps_cls", bufs=2, space="PSUM") as ps_cls,
            tc.tile_pool(name="ps_g", bufs=1, space="PSUM") as ps_g,
            tc.tile_pool(name="ps_t", bufs=2, space="PSUM") as ps_t,
            tc.tile_pool(name="dram", bufs=1, space="DRAM") as dram,
        ):
            # ---- resident weights ----
            wihT = wpool.tile([128, 2 * 3 * H], F32)          # 2 chunks of [128,1536]
            for k in range(2):
                nc.sync.dma_start(wihT[:, k * 1536:(k + 1) * 1536],
                                  wihT_d[128 * k:128 * (k + 1), :])
            whhT = wpool.tile([128, 4 * 3 * H], F32)          # 4 chunks
            for k in range(4):
                nc.sync.dma_start(whhT[:, k * 1536:(k + 1) * 1536],
                                  whhT_d[128 * k:128 * (k + 1), :])
            wcT = wpool.tile([128, 4 * VS], F32)              # 4 chunks of [128,6656]
            for k in range(4):
                nc.sync.dma_start(wcT[:, k * VS:(k + 1) * VS],
                                  wcT_d[128 * k:128 * (k + 1), :])
            bc2 = wpool.tile([128, HALF0], F32)
            nc.sync.dma_start(bc2[:], bc2_d[:])
            brz = wpool.tile([B, 2 * H], F32)
            nc.sync.dma_start(brz[:], brz_d[:])
            bihn = wpool.tile([B, H], F32)
            nc.sync.dma_start(bihn[:], bihn_d[:])
            bhhn = wpool.tile([B, H], F32)
            nc.sync.dma_start(bhhn[:], bhhn_d[:])
            off128 = wpool.tile([128, 1], F32)
            nc.sync.dma_start(off128[:], off_d[:])
            ident = wpool.tile([128, 128], F32)
            make_identity(nc, ident[:])

            # ---- persistent state ----
            h = state.tile([B, H], F32)
            nc.sync.dma_start(h[:], h0_d[:])
            hT = state.tile([128, 4 * B], F32)                # 4 chunks of [128,64]
            for k in range(4):
                nc.sync.dma_start(hT[:, B * k:B * (k + 1)],
                                  h0T_d[128 * k:128 * (k + 1), :])
            tok = state.tile([B, 1], I32)
            nc.vector.memset(tok[:], SOS)

            sb_logits = lgt.tile([128, HALF0], F32)
            # rows 64:128, cols HALF1:HALF0 are never written per-step; make
            # them a -inf pad so the max scan can cover the full [128, 3584].
            nc.vector.memset(sb_logits[64:128, HALF1:HALF0], NEG)

            agr = state.tile([N_CORES, 256], F32)
            if sim_local:
                nc.vector.memset(agr[1:N_CORES, :], NEG)

            for t in range(n_steps):
                if not sim_local:
                    # AG bounce buffers (DRAM) -- shared output needs one
                    # writer, so allocate per step
                    ag_in = dram.tile([1, 256], F32, tag=f"agi{t}")
                    ag_out_sh = dram.tile([N_CORES, 256], F32,
                                          addr_space="Shared", tag=f"ago{t}")

                # ---- 1. embedding gather: x[b] = emb[tok[b]] ----
                x = work.tile([B, E], F32, tag="x")
                nc.gpsimd.indirect_dma_start(
                    out=x[:], out_offset=None, in_=emb_d[:],
                    in_offset=bass.IndirectOffsetOnAxis(ap=tok[:, :1], axis=0),
                )
                # xT chunks [128, 64] x2 via PE transpose
                xT = work.tile([128, 2 * B], F32, tag="xT")
                for j in range(2):
                    pt = ps_t.tile([128, B], F32, tag="pst")
                    nc.tensor.transpose(out=pt[:], in_=x[:, 128 * j:128 * (j + 1)],
                                        identity=ident[0:B, 0:B])
                    nc.vector.tensor_copy(xT[:, B * j:B * (j + 1)], pt[:])

                # ---- 2. GRU matmuls ----
                # psum_g[:, 0:1024]  = gi_rz + gh_rz ; psum_g[:, 1024:1536] = gi_n
                # psum_hn            = gh_n
                pg = ps_g.tile([B, 3 * H], F32, tag="pg")
                phn = ps_g.tile([B, H], F32, tag="phn")
                for seg in range(3):
                    sl = slice(512 * seg, 512 * (seg + 1))
                    for k in range(2):
                        nc.tensor.matmul(
                            pg[:, sl], lhsT=xT[:, B * k:B * (k + 1)],
                            rhs=wihT[:, 1536 * k + 512 * seg:1536 * k + 512 * (seg + 1)],
                            start=(k == 0), stop=(seg == 2 and k == 1),
                        )
                    if seg < 2:
                        for k in range(4):
                            nc.tensor.matmul(
                                pg[:, sl], lhsT=hT[:, B * k:B * (k + 1)],
                                rhs=whhT[:, 1536 * k + 512 * seg:1536 * k + 512 * (seg + 1)],
                                start=False, stop=(k == 3),
                            )
                for k in range(4):
                    nc.tensor.matmul(
                        phn[:], lhsT=hT[:, B * k:B * (k + 1)],
                        rhs=whhT[:, 1536 * k + 1024:1536 * k + 1536],
                        start=(k == 0), stop=(k == 3),
                    )

                # ---- 3. GRU elementwise ----
                rz_pre = work.tile([B, 2 * H], F32, tag="rzp")
                nc.vector.tensor_add(rz_pre[:], pg[:, 0:1024], brz[:])
                rz = work.tile([B, 2 * H], F32, tag="rz")
                nc.scalar.activation(rz[:], rz_pre[:],
                                     mybir.ActivationFunctionType.Sigmoid)
                hn_b = work.tile([B, H], F32, tag="hnb")
                nc.vector.tensor_add(hn_b[:], phn[:], bhhn[:])
                tmp = work.tile([B, H], F32, tag="tmp")
                nc.vector.tensor_mul(tmp[:], rz[:, 0:512], hn_b[:])
                in_b = work.tile([B, H], F32, tag="inb")
                nc.vector.tensor_add(in_b[:], pg[:, 1024:1536], bihn[:])
                tmp2 = work.tile([B, H], F32, tag="tmp2")
                nc.vector.tensor_add(tmp2[:], tmp[:], in_b[:])
                ngate = work.tile([B, H], F32, tag="ngate")
                nc.scalar.activation(ngate[:], tmp2[:],
                                     mybir.ActivationFunctionType.Tanh)
                d = work.tile([B, H], F32, tag="d")
                nc.vector.tensor_sub(d[:], h[:], ngate[:])
                zd = work.tile([B, H], F32, tag="zd")
                nc.vector.tensor_mul(zd[:], rz[:, 512:1024], d[:])
                nc.vector.tensor_add(h[:], ngate[:], zd[:])

                # hT update: 4 PE transposes of h chunks
                for k in range(4):
                    pt = ps_t.tile([128, B], F32, tag="pst")
                    nc.tensor.transpose(out=pt[:], in_=h[:, 128 * k:128 * (k + 1)],
                                        identity=ident[0:B, 0:B])
                    nc.vector.tensor_copy(hT[:, B * k:B * (k + 1)], pt[:])

                # ---- 4. classifier: 7 chunk pairs (p=6: half0 solo) ----
                for p in range(7):
                    ps = ps_cls.tile([128, 512], F32, tag="cls")
                    for k in range(4):
                        nc.tensor.matmul(
                            ps[0:64, :], lhsT=hT[:, B * k:B * (k + 1)],
                            rhs=wcT[:, VS * k + 512 * p:VS * k + 512 * (p + 1)],
                            start=(k == 0), stop=(k == 3), tile_position=(0, 0),
                        )
                    if p < 6:
                        for k in range(4):
                            nc.tensor.matmul(
                                ps[64:128, :], lhsT=hT[:, B * k:B * (k + 1)],
                                rhs=wcT[:, VS * k + HALF0 + 512 * p:
                                        VS * k + HALF0 + 512 * (p + 1)],
                                start=(k == 0), stop=(k == 3), tile_position=(0, 64),
                            )
                        nc.vector.tensor_add(
                            sb_logits[:, 512 * p:512 * (p + 1)], ps[:],
                            bc2[:, 512 * p:512 * (p + 1)])
                    else:
                        nc.vector.tensor_add(
                            sb_logits[0:64, 512 * p:512 * (p + 1)], ps[0:64, :],
                            bc2[0:64, 512 * p:512 * (p + 1)])

                # ---- 5. DMA logits out ----
                nc.sync.dma_start(out_d[t, :, 0:HALF0], sb_logits[0:64, :])
                nc.sync.dma_start(out_d[t, :, HALF0:VS], sb_logits[64:128, 0:HALF1])
                nc.sync.dma_start(toks_d[t], tok[:])

                # ---- 6. local argmax candidates ----
                m8 = work.tile([128, 8], F32, tag="m8")
                mi8 = work.tile([128, 8], U32, tag="mi8")
                nc.vector.max(out=m8[:], in_=sb_logits[:])
                nc.vector.max_index(out=mi8[:], in_max=m8[:], in_values=sb_logits[:])
                vi = work.tile([128, 2], F32, tag="vi")
                nc.vector.tensor_copy(vi[:, 0:1], m8[:, 0:1])
                idxf = work.tile([128, 1], F32, tag="idxf")
                nc.vector.tensor_copy(idxf[:], mi8[:, 0:1])      # u32 -> f32
                nc.vector.tensor_add(vi[:, 1:2], idxf[:], off128[:])

                # ---- 7. AllGather of (val, idx) ----
                pvt = ps_t.tile([2, 128], F32, tag="pst")
                nc.tensor.transpose(out=pvt[:], in_=vi[:], identity=ident[:])
                agi = work.tile([2, 128], F32, tag="agi")
                nc.vector.tensor_copy(agi[:], pvt[:])
                if sim_local:
                    # timing-only stand-in for the collective round trip
                    nc.sync.dma_start(agr[0:1, :], agi[:])
                else:
                    nc.sync.dma_start(ag_in[:], agi[:])
                    nc.gpsimd.collective_compute(
                        "AllGather", mybir.AluOpType.bypass,
                        replica_groups=[list(range(N_CORES))],
                        ins=[ag_in.opt()], outs=[ag_out_sh.opt()],
                    )
                    nc.sync.dma_start(agr[:], ag_out_sh[:])

                # ---- 8. lane-aligned global combine ----
                # agr row c = [vA 64 | vB 64 | iA 64 | iB 64] from core c.
                # 4 transposes -> cand [64, 32] = [vA 8 | vB 8 | iA 8 | iB 8]
                pcand = ps_t.tile([B, 32], F32, tag="pst")
                for q in range(4):
                    nc.tensor.transpose(out=pcand[:, 8 * q:8 * (q + 1)],
                                        in_=agr[:, 64 * q:64 * (q + 1)],
                                        identity=ident[0:N_CORES, 0:N_CORES])
                cand = work.tile([B, 32], F32, tag="cand")
                nc.vector.tensor_copy(cand[:], pcand[:])
                gmax = work.tile([B, 1], F32, tag="gmax")
                nc.vector.tensor_reduce(gmax[:], cand[:, 0:16],
                                        axis=mybir.AxisListType.X,
                                        op=mybir.AluOpType.max)
                mask = work.tile([B, 16], F32, tag="mask")
                nc.vector.tensor_scalar(out=mask[:], in0=cand[:, 0:16],
                                        scalar1=gmax[:, 0:1], scalar2=None,
                                        op0=mybir.AluOpType.is_ge)
                idxmb = work.tile([B, 16], F32, tag="idxmb")
                nc.vector.tensor_scalar(out=idxmb[:], in0=cand[:, 16:32],
                                        scalar1=BIG, scalar2=None,
                                        op0=mybir.AluOpType.subtract)
                sel = work.tile([B, 16], F32, tag="sel")
                nc.vector.tensor_mul(sel[:], mask[:], idxmb[:])
                mn = work.tile([B, 1], F32, tag="mn")
                nc.vector.tensor_reduce(mn[:], sel[:],
                                        axis=mybir.AxisListType.X,
                                        op=mybir.AluOpType.min)
                tokf = work.tile([B, 1], F32, tag="tokf")
                # token = min(mn + BIG, V-1): clamp guards the emb gather
                nc.vector.tensor_scalar(out=tokf[:], in0=mn[:], scalar1=BIG,
                                        scalar2=float(V - 1),
                                        op0=mybir.AluOpType.add,
                                        op1=mybir.AluOpType.min)
                nc.vector.tensor_copy(tok[:], tokf[:])           # f32 -> i32

    nc.compile()
    return nc


def prep_inputs(h0, emb, W_ih, W_hh, b_ih, b_hh, Wc, bc):
    """Host-side numpy prep. Returns list of per-core input dicts."""
    h0 = np.asarray(h0, np.float32)
    emb = np.ascontiguousarray(np.asarray(emb, np.float32))
    W_ih = np.asarray(W_ih, np.float32)
    W_hh = np.asarray(W_hh, np.float32)
    b_ih = np.asarray(b_ih, np.float32)
    b_hh = np.asarray(b_hh, np.float32)
    Wc = np.asarray(Wc, np.float32)
    bc = np.asarray(bc, np.float32)

    wihT = np.ascontiguousarray(W_ih.T)                  # [256, 1536]
    whhT = np.ascontiguousarray(W_hh.T)                  # [512, 1536]
    brz = np.broadcast_to((b_ih + b_hh)[None, 0:1024], (B, 1024)).copy()
    bihn = np.broadcast_to(b_ih[None, 1024:1536], (B, 512)).copy()
    bhhn = np.broadcast_to(b_hh[None, 1024:1536], (B, 512)).copy()
    h0s = np.ascontiguousarray(h0[0])                    # [64, 512]
    h0T = np.ascontiguousarray(h0s.T)                    # [512, 64]

    in_maps = []
    for c in range(N_CORES):
        base = c * W_BASE
        w_c = shard_width(c)
        wcT = np.zeros((H, VS), np.float32)
        wcT[:, :w_c] = Wc[base:base + w_c].T
        bcp = np.full(VS, NEG, np.float32)
        bcp[:w_c] = bc[base:base + w_c]
        bc2 = np.empty((128, HALF0), np.float32)
        bc2[0:64, :] = bcp[0:HALF0][None, :]
        bc2[64:128, 0:HALF1] = bcp[HALF0:VS][None, :]
        bc2[64:128, HALF1:HALF0] = NEG
        off = np.empty((128, 1), np.float32)
        off[0:64] = base
        off[64:128] = base + HALF0
        in_maps.append(dict(
            emb=emb, wihT=wihT, whhT=whhT, wcT=np.ascontiguousarray(wcT),
            bc2=bc2, brz=brz, bihn=bihn, bhhn=bhhn, off128=off,
            h0s=h0s, h0T=h0T,
        ))
    return in_maps


_NC_CACHE = {}


def _get_nc(n_steps):
    if n_steps not in _NC_CACHE:
        _NC_CACHE[n_steps] = build_kernel(n_steps)
    return _NC_CACHE[n_steps]


def run(inputs, n_steps=L, trace=False):
    in_maps = prep_inputs(
        inputs["h0"], inputs["emb"], inputs["W_ih"], inputs["W_hh"],
        inputs["b_ih"], inputs["b_hh"], inputs["Wc"], inputs["bc"])
    nc = _get_nc(n_steps)
    res = run_bass_kernel_spmd(nc, in_maps, core_ids=list(range(N_CORES)),
                               trace=trace)
    parts = []
    for c in range(N_CORES):
        parts.append(res.results[c]["out"][:, :, :shard_width(c)])
    full = np.concatenate(parts, axis=2)
    toks = res.results[0]["toks"][:, :, 0]
    return full, toks, res


def kernel(**inputs) -> np.ndarray:
    n_steps = int(inputs.get("len_seq", L))
    full, _toks, _res = run(inputs, n_steps=n_steps)
    return full


if __name__ == "__main__":
    pass

